# revision 31
# baseline (speedup 1.0000x reference)
"""Bass/Trainium2 kernel for the BayesTensorRing embedding-lookup problem.

out[i] = trace( prod_d (core_d[idx_d] * diag(lam_d)) ),  N=500k rows, 8 cores.

Strategy:
  * Host (one-time weight repacking, independent of the index stream):
    fold each lambda into its core, then build two pair-product tables
       T1[(j0,j1)][x,z] = sum_y A0[j0,x,y] A1[j1,y,z]     (A0A1)
       T2[(j2,j3)][x,z] = sum_w A2[j2,z,w] A3[j3,w,x]     ((A2A3)^T layout)
    each [40000, 256].  out[i] = <T1[p_i], T2[q_i]>, p = i0*200+i1,
    q = i2*200+i3.  A shared Hadamard rotation (invariant under T1 H, T2 H)
    plus per-table scaling quantizes both tables to fp8-e3m4 (256 B rows);
    the host folds the inverse scale back into the output.
  * Device (all per-row work): shard N over 8 NeuronCores.  The tile
    scheduler rotates only 8 SWDGE DMA semaphores, which caps outstanding
    gathers — so both table reads for a 512-row chunk are fused into ONE
    1024-index dma_gather: the host concatenates quarter-tables
    C[a,b] = [T1-quarter-a ; T2-quarter-b] (16 combos, 20032 rows each, all
    int16-addressable) and emits the index stream [p-idxs | q-idxs].  Rows
    are bucketed by (p//10016, q//10016) so each chunk uses one combo table.
    Pad indices are spread over a ramp (a constant pad row would serialize
    one HBM bank).  Per chunk: one gather (single_packet=False for finer
    SDMA round-robin), one DVE fp8 multiply of the p-half against the
    q-half, then free-dim reductions split between the scalar/ACT engine
    (activation-Copy accumulators) and DVE to balance engine load.
"""

import sys

import numpy as np

sys.path.insert(0, "/opt/trn_rl_repo")

from concourse import bacc, mybir
import concourse.tile as tile
from concourse.bass_utils import run_bass_kernel_spmd

NCORES = 8
DIM = 200
R = 16
RR = R * R  # 256 (row elements; 256 B in e3m4)
NTAB = DIM * DIM  # 40000
QS = 10016  # quarter size (4*QS = 40064 >= NTAB, padded)
NTAB_PAD = 4 * QS
CTAB = 2 * QS  # rows per combined quarter-pair table
HALF = 32768
N = 500_000
P = 128
CHUNK = 512  # output rows per chunk (one 1024-idx gather: 512 p + 512 q)
GIDX = 2 * CHUNK  # gather indices per chunk (ucode max 1024)
SUB = CHUNK // P  # 4 sub-tiles of 128 rows per chunk
ACT_SUBS = 2  # sub-reductions offloaded to the scalar/ACT engine (rest DVE)
IW = GIDX // 16  # idx words per chunk per partition


def _hadamard(n):
    H = np.array([[1.0]], np.float64)
    while H.shape[0] < n:
        H = np.block([[H, H], [H, -H]])
    return (H / np.sqrt(n)).astype(np.float32)


def _tables(core0, core1, core2, core3, lam0, lam1, lam2, lam3):
    """e3m4 tables. Shared Hadamard rotation (invariant for the dot) spreads
    outliers; per-table scale maximizes e3m4 range use. Returns (T1,T2,inv_scale)."""
    import ml_dtypes

    A0 = (core0 * lam0[None, None, :]).astype(np.float32)
    A1 = (core1 * lam1[None, None, :]).astype(np.float32)
    A2 = (core2 * lam2[None, None, :]).astype(np.float32)
    A3 = (core3 * lam3[None, None, :]).astype(np.float32)
    # T1[(j0,j1)][x,z] = sum_y A0[j0,x,y] A1[j1,y,z]
    M1 = A0.reshape(DIM * R, R) @ np.ascontiguousarray(A1.transpose(1, 0, 2)).reshape(
        R, DIM * R
    )  # [(j0 x), (j1 z)]
    T1 = np.ascontiguousarray(
        M1.reshape(DIM, R, DIM, R).transpose(0, 2, 1, 3)
    ).reshape(NTAB, RR)
    # T2[(j2,j3)][x,z] = sum_w A2[j2,z,w] A3[j3,w,x]
    M2 = A2.reshape(DIM * R, R) @ np.ascontiguousarray(A3.transpose(1, 0, 2)).reshape(
        R, DIM * R
    )  # [(j2 z), (j3 x)]
    T2 = np.ascontiguousarray(
        M2.reshape(DIM, R, DIM, R).transpose(0, 2, 3, 1)
    ).reshape(NTAB, RR)
    H = _hadamard(RR)
    T1 = T1 @ H
    T2 = T2 @ H
    s1 = 12.0 / max(np.abs(T1).max(), 1e-30)
    s2 = 12.0 / max(np.abs(T2).max(), 1e-30)
    T1q = (T1 * s1).astype(ml_dtypes.float8_e3m4)
    T2q = (T2 * s2).astype(ml_dtypes.float8_e3m4)
    return T1q, T2q, 1.0 / (s1 * s2)


def _combined_table(T1q, T2q):
    """[16*CTAB, RR]: combo (a,b) holds [T1 quarter a ; T2 quarter b]."""
    t1p = np.zeros((NTAB_PAD, RR), dtype=T1q.dtype)
    t1p[:NTAB] = T1q
    t2p = np.zeros((NTAB_PAD, RR), dtype=T2q.dtype)
    t2p[:NTAB] = T2q
    parts = []
    for a in range(4):
        for b in range(4):
            parts.append(t1p[a * QS : (a + 1) * QS])
            parts.append(t2p[b * QS : (b + 1) * QS])
    return np.ascontiguousarray(np.concatenate(parts, axis=0))


def _wrap16(a):
    """Pack a [n*16k] idx list into dma_gather layout [128, n]:
    idx i -> [i % 16, i // 16], replicated over the 8 partition groups."""
    w = np.ascontiguousarray(a.reshape(-1, 16).T)  # [16, len/16]
    return np.ascontiguousarray(np.tile(w, (8, 1)))  # [128, len/16]


def build_program(flavors):
    """flavors: tuple of (a, b) quarter pairs per chunk — static combo table."""
    nch = len(flavors)
    nc = bacc.Bacc("TRN2", target_bir_lowering=False, num_swdge_queues=4,
                   dynamic_dma_scratch_size=131072)
    f16 = mybir.dt.float16
    f32 = mybir.dt.float32
    f8 = mybir.dt.float8e3
    i16 = mybir.dt.int16

    tc_d = nc.dram_tensor("tc", [16 * CTAB, RR], f8, kind="ExternalInput")
    cidx = nc.dram_tensor("cidx", [P, nch * IW], i16, kind="ExternalInput")
    out = nc.dram_tensor("out", [P, nch * SUB], f32, kind="ExternalOutput")

    with tile.TileContext(nc) as tc:
        with (
            tc.tile_pool(name="idx", bufs=8) as idx_pool,
            tc.tile_pool(name="g1", bufs=8) as g_pool,
            tc.tile_pool(name="prod", bufs=6) as prod_pool,
            tc.tile_pool(name="scr", bufs=4) as scr_pool,
            tc.tile_pool(name="res", bufs=1) as res_pool,
        ):
            out_sb = res_pool.tile([P, nch * SUB], f32)

            for c, (a, b) in enumerate(flavors):
                combo = a * 4 + b
                src = tc_d[combo * CTAB : (combo + 1) * CTAB, :]
                c_sb = idx_pool.tile([P, IW], i16, tag="c")
                nc.sync.dma_start(out=c_sb[:], in_=cidx[:, c * IW : (c + 1) * IW])
                g = g_pool.tile([P, 2 * SUB, RR], f8)
                nc.gpsimd.dma_gather(
                    g[:], src, c_sb[:], GIDX, GIDX, RR,
                    queue_num=c % 4, single_packet=False,
                )
                prod = prod_pool.tile([P, SUB, RR], f16)
                nc.vector.tensor_tensor(
                    out=prod[:], in0=g[:, :SUB, :], in1=g[:, SUB:, :],
                    op=mybir.AluOpType.mult,
                )
                # reduce: ACT engine takes ACT_SUBS sub-tiles (activation-Copy
                # with running accumulator = free-dim sum), DVE the rest.
                scr = scr_pool.tile([P, RR], f16, tag="scr")
                for s in range(ACT_SUBS):
                    nc.scalar.activation(
                        out=scr[:],
                        in_=prod[:, s, :],
                        func=mybir.ActivationFunctionType.Copy,
                        accum_out=out_sb[:, c * SUB + s : c * SUB + s + 1],
                    )
                if ACT_SUBS < SUB:
                    nc.vector.tensor_reduce(
                        out=out_sb[:, c * SUB + ACT_SUBS : (c + 1) * SUB],
                        in_=prod[:, ACT_SUBS:, :],
                        axis=mybir.AxisListType.X,
                        op=mybir.AluOpType.add,
                    )
            nc.sync.dma_start(out=out[:], in_=out_sb[:])
    nc.compile()
    return nc


_PROG_CACHE = {}


def _get_program(flavors):
    key = tuple(flavors)
    if key not in _PROG_CACHE:
        _PROG_CACHE[key] = build_program(key)
    return _PROG_CACHE[key]


def plan(index):
    """Bucket rows per core by quarter pair, build a shared chunk grid."""
    idx = np.asarray(index).astype(np.int64)
    p_all = (idx[:, 0] * DIM + idx[:, 1]).astype(np.int32)
    q_all = (idx[:, 2] * DIM + idx[:, 3]).astype(np.int32)
    rows = N // NCORES
    per_core = []
    counts = np.zeros((NCORES, 16), np.int64)
    for c in range(NCORES):
        sl = slice(c * rows, (c + 1) * rows)
        p, q = p_all[sl], q_all[sl]
        bkt = (p // QS) * 4 + (q // QS)
        ids = [np.where(bkt == bb)[0] for bb in range(16)]
        counts[c] = [len(x) for x in ids]
        per_core.append((p, q, ids))
    nch_b = [max(1, -(-int(counts[:, bb].max()) // CHUNK)) for bb in range(16)]
    flavors = []
    for bb in range(16):
        flavors += [(bb >> 2, bb & 3)] * nch_b[bb]
    return per_core, nch_b, tuple(flavors)


def make_in_maps(per_core, nch_b, TC):
    in_maps, metas = [], []
    for p, q, ids in per_core:
        chunks, meta = [], []
        for bb in range(16):
            a, b = bb >> 2, bb & 3
            cap = nch_b[bb] * CHUNK
            sel = ids[bb]
            # spread pad indices over the quarter: a constant pad row would
            # hammer one HBM bank with identical reads per pad chunk.
            ramp = np.arange(cap, dtype=np.int32) % 9000
            pd = ramp.copy()
            qd = QS + ramp
            pd[: len(sel)] = p[sel] - a * QS
            qd[: len(sel)] = QS + (q[sel] - b * QS)
            for i in range(nch_b[bb]):
                chunks.append(np.concatenate(
                    [pd[i * CHUNK : (i + 1) * CHUNK],
                     qd[i * CHUNK : (i + 1) * CHUNK]]
                ).astype(np.int16))
            meta.append(sel)
        cw = np.concatenate([_wrap16(ch) for ch in chunks], axis=1)
        in_maps.append({"tc": TC, "cidx": cw})
        metas.append(meta)
    return in_maps, metas


def unpack(results, metas, nch_b, inv_scale):
    rows = N // NCORES
    outs = []
    for c in range(NCORES):
        o = results[c]["out"]  # [128, nch*SUB]; row i of chunk -> [i%128, i//128]
        flat = np.asarray(o).T.reshape(-1) * inv_scale  # (chunk, sub, part) order
        full = np.empty(rows, np.float32)
        off = 0
        for bb in range(16):
            cap = nch_b[bb] * CHUNK
            sel = metas[c][bb]
            full[sel] = flat[off : off + len(sel)]
            off += cap
        outs.append(full)
    return np.concatenate(outs)


def kernel(index, core0, core1, core2, core3, lam0, lam1, lam2, lam3, _trace=False):
    T1, T2, inv_scale = _tables(
        np.asarray(core0), np.asarray(core1), np.asarray(core2), np.asarray(core3),
        np.asarray(lam0), np.asarray(lam1), np.asarray(lam2), np.asarray(lam3),
    )
    per_core, nch_b, flavors = plan(index)
    nc = _get_program(flavors)
    in_maps, metas = make_in_maps(per_core, nch_b, _combined_table(T1, T2))
    res = run_bass_kernel_spmd(
        nc, in_maps, core_ids=list(range(NCORES)), trace=_trace
    )
    full = unpack(res.results, metas, nch_b, inv_scale).astype(np.float32)
    if _trace:
        return full, res
    return full
